# revision 3
# baseline (speedup 1.0000x reference)
"""Trainium2 Bass kernel for the nn_AaD retrieval-KNN loss (v4.1: residue fold).

Self-contained: takes the FULL unsharded inputs, shards fea_bank row-wise
across 8 NeuronCores. Per core the Bass program does:
  - fp8 DoubleRow distance matmuls (features stationary, K=256 per step),
    13 column groups of 512, two 128-row batch halves (m).
  - fbt is DMA'd in 7 pair-aligned chunks whose descriptor generation is
    split across BOTH HWDGE sequencers (sync + scalar) so HBM streams at
    line rate from kernel start.
  - drain+reduce per m: the 13 group distances [128, 512] fp32 in PSUM are
    max-folded into one 512-wide bf16 "residue max" per batch row:
      * pair0 (g0,g1) and the last group g12 drain on the vector engine
        (cast / tensor_tensor-max straight from PSUM into an accumulator),
      * pairs 1-5 (g2..g11) are copied PSUM->SBUF bf16 by the scalar
        engine, then folded by wide 2x-mode tensor_tensor max ops on the
        vector engine (batched 8-slot tree mid-stream, so only ~2us of
        fold work remains after the last matmul),
    residues are halved to 256, then MAX8 + FIND_INDEX8 give the top-8
    residue ids per batch row, DMA'd out once.
The top-6 distances of any row provably live inside that core's top-8
residues (each residue pools 26 columns: col = g*512 + h*256 + r), so the
host rescans the winning residues in exact fp32, re-ranks with lax.top_k
tie-breaking, and computes the KL + dispersion loss in numpy.
"""

import numpy as np
import ml_dtypes

import concourse.mybir as mybir
import concourse.tile as tile
from concourse import bacc
from concourse.bass_utils import run_bass_kernel_spmd

B, D, C, N, K = 256, 512, 345, 50000, 5
ALPHA = 1.0
EPS = 1e-12
M = 8                   # cores
NS = N // M             # 6250 bank rows per core
G = 13                  # 512-wide column groups per core
GW = 512
NPAD = G * GW           # 6656
R = 256                 # final residue count per batch row
PAIRS = [(0, 2), (2, 4), (4, 6), (6, 8), (8, 10), (10, 12), (12, 13)]
SYNC_CHUNKS = [0, 2, 4, 6]      # pair indices DMA'd from the sync engine
SCAL_CHUNKS = [1, 3, 5]         # pair indices DMA'd from the scalar engine

F32 = mybir.dt.float32
BF16 = mybir.dt.bfloat16
F8 = mybir.dt.float8e4
U32 = mybir.dt.uint32
AF = mybir.ActivationFunctionType
ALU = mybir.AluOpType
DR = mybir.MatmulPerfMode.DoubleRow

_CACHE: dict = {}


def _build():
    nc = bacc.Bacc("TRN2", target_bir_lowering=False, debug=False, num_devices=M)

    # fbt[p, g, dk, c] = fb_slab.T[dk*128+p, g*512+c]
    fbt_in = nc.dram_tensor("fbt", [128, G, 4, GW], F8, kind="ExternalInput")
    # fnt[p, dk, m] = fn[m, dk*128+p]
    fnt_in = nc.dram_tensor("fnt", [128, 4, B], F8, kind="ExternalInput")
    out_idx = nc.dram_tensor("out_idx", [128, 2, 8], U32, kind="ExternalOutput")
    junk_out = nc.dram_tensor("junk_out", [1, 8], F32, kind="ExternalOutput")

    with tile.TileContext(nc) as tc:
        with (
            tc.tile_pool(name="const", bufs=1) as constp,
            tc.tile_pool(name="small", bufs=2) as smallp,
            tc.tile_pool(name="psum", bufs=3, space="PSUM") as psp,
            tc.tile_pool(name="psumj", bufs=1, space="PSUM") as pspj,
        ):
            fnt_sb = constp.tile([128, 4, B], F8, tag="fnt")
            fbt_sb = constp.tile([128, G, 4, GW], F8, tag="fbt")

            # descriptor generation is ~650ns per dma_start and serializes
            # per sequencer: split the chunk issues across sync + scalar
            nc.sync.dma_start(fnt_sb[:], fnt_in[:])
            for pi in SYNC_CHUNKS:
                ga, gb = PAIRS[pi]
                nc.sync.dma_start(fbt_sb[:, ga:gb], fbt_in[:, ga:gb])
            for pi in SCAL_CHUNKS:
                ga, gb = PAIRS[pi]
                nc.scalar.dma_start(fbt_sb[:, ga:gb], fbt_in[:, ga:gb])

            # PE warm-up: dummy matmuls on a locally-initialized tile keep
            # TensorE busy while the first fbt chunk is still in flight.
            junk_src = constp.tile([128, GW], BF16, tag="junksrc")
            nc.vector.memset(junk_src[:], 1.0)
            junk_ps = pspj.tile([128, GW], F32, tag="junk")
            NJUNK = 4
            for wi in range(NJUNK):
                nc.tensor.matmul(junk_ps[:], lhsT=junk_src[:, 0:128],
                                 rhs=junk_src[:], start=(wi == 0),
                                 stop=(wi == NJUNK - 1))
            junk_sb = constp.tile([1, 8], F32, tag="junksb")
            nc.scalar.activation(junk_sb[:], junk_ps[:1, :8], AF.Copy)
            nc.scalar.dma_start(junk_out[:], junk_sb[:])

            # per-m working areas; slots is a flat bf16 scratch row:
            #   [0:5120)     slots 0..9  (ACT drains of pairs 1-5)
            #   [5120:7168)  fold 8->4 output
            #   [7168:8192)  fold 4->2 output
            #   [5120:5632)  fold 2->1 output S1 (reuse)
            #   [5632:6144)  slot8+slot9 fold s89
            #   [6144:6656)  S2 = max(S1, s89)
            slots = [constp.tile([128, 8192], BF16, tag=f"slots{m}",
                                 name=f"slots{m}") for m in range(2)]
            acc = [constp.tile([128, GW], BF16, tag=f"acc{m}",
                               name=f"acc{m}") for m in range(2)]
            res = [constp.tile([128, R], BF16, tag=f"res{m}",
                               name=f"res{m}") for m in range(2)]
            sel8 = constp.tile([128, 2, 8], U32, tag="sel8")

            for pi, (ga, gb) in enumerate(PAIRS):
                for m in range(2):
                    pt = psp.tile([128, 2, GW], F32, tag="pp",
                                  name=f"pp{(pi * 2 + m) % 3}")
                    for kc in range(2):
                        for g in range(ga, gb):
                            nc.tensor.matmul(
                                pt[:, g - ga],
                                lhsT=fnt_sb[:, 2 * kc:2 * kc + 2,
                                            m * 128:(m + 1) * 128],
                                rhs=fbt_sb[:, g, 2 * kc:2 * kc + 2, :],
                                start=(kc == 0),
                                stop=(kc == 1),
                                perf_mode=DR,
                            )
                    s = slots[m]
                    if pi == 0:
                        # vector engine drains pair0: init acc, fold g1 in
                        nc.vector.tensor_copy(acc[m][:], pt[:, 0])
                        nc.vector.tensor_tensor(acc[m][:], pt[:, 1], acc[m][:],
                                                ALU.max)
                    elif pi < 6:
                        # scalar engine drains pairs 1-5 into slots 0..9
                        j = pi - 1
                        nc.scalar.activation(
                            s[:, j * 1024:(j + 1) * 1024], pt[:], AF.Copy)
                    else:
                        # last single group g12: fold straight into acc
                        nc.vector.tensor_tensor(acc[m][:], pt[:, 0], acc[m][:],
                                                ALU.max)
                    if pi == 4:
                        # slots 0..7 ready: batched fold tree 8 -> 1 (= S1)
                        nc.vector.tensor_tensor(
                            s[:, 5120:7168], s[:, 0:2048], s[:, 2048:4096],
                            ALU.max)                      # 8 -> 4
                        nc.vector.tensor_tensor(
                            s[:, 7168:8192], s[:, 5120:6144], s[:, 6144:7168],
                            ALU.max)                      # 4 -> 2
                        nc.vector.tensor_tensor(
                            s[:, 5120:5632], s[:, 7168:7680], s[:, 7680:8192],
                            ALU.max)                      # 2 -> 1 = S1
                    if pi == 5:
                        # slots 8,9 ready: finish the slot tree (= S2)
                        nc.vector.tensor_tensor(
                            s[:, 5632:6144], s[:, 4096:4608], s[:, 4608:5120],
                            ALU.max)                      # s89
                        nc.vector.tensor_tensor(
                            s[:, 6144:6656], s[:, 5120:5632], s[:, 5632:6144],
                            ALU.max)                      # S2
                    if pi == 6:
                        nc.vector.tensor_tensor(acc[m][:], s[:, 6144:6656],
                                                acc[m][:], ALU.max)
                        nc.vector.tensor_tensor(res[m][:], acc[m][:, 0:R],
                                                acc[m][:, R:GW], ALU.max)
                        mx8 = smallp.tile([128, 8], BF16, tag=f"mx8_{m}",
                                          name=f"mx8_{m}")
                        nc.vector.max(out=mx8[:], in_=res[m][:])
                        nc.vector.max_index(out=sel8[:, m], in_max=mx8[:],
                                            in_values=res[m][:])
            nc.sync.dma_start(out_idx[:], sel8[:])

    nc.compile()
    return nc


def _get_nc():
    if "nc" not in _CACHE:
        _CACHE["nc"] = _build()
    return _CACHE["nc"]


def _prep(features, predictions, fea_bank, score_bank, trg_idx):
    feat = np.asarray(features, dtype=np.float32)
    pred = np.asarray(predictions, dtype=np.float32)
    fb = np.array(fea_bank, dtype=np.float32)
    sb = np.array(score_bank, dtype=np.float32)
    trg = np.asarray(trg_idx).astype(np.int64)

    x = pred - pred.max(axis=1, keepdims=True)
    e = np.exp(x)
    p = e / e.sum(axis=1, keepdims=True)

    nrm = np.sqrt((feat * feat).sum(axis=1, keepdims=True))
    fn = feat / np.maximum(nrm, EPS)

    fb[trg] = fn
    sb[trg] = p

    fnt = np.ascontiguousarray(
        fn.T.reshape(4, 128, B).transpose(1, 0, 2)).astype(ml_dtypes.float8_e4m3)

    in_maps = []
    for c in range(M):
        slabT = np.zeros((D, NPAD), dtype=np.float32)
        slabT[:, :NS] = fb[c * NS:(c + 1) * NS].T
        fbt = np.ascontiguousarray(
            slabT.reshape(4, 128, G, GW).transpose(1, 2, 0, 3)
        ).astype(ml_dtypes.float8_e4m3)
        in_maps.append({"fbt": fbt, "fnt": fnt})
    return in_maps, fn, fb, sb, p


def _merge(results, fn, fb, sb, p):
    # residue r of half m covers local padded columns g*512 + h*256 + r
    base = (np.arange(G)[:, None] * GW
            + np.arange(2)[None, :] * R).reshape(-1)     # [26]
    gls, vas = [], []
    for c in range(M):
        sel = results[c]["out_idx"].reshape(128, 2, 8)
        sel = np.concatenate([sel[:, 0], sel[:, 1]], axis=0).astype(np.int64)
        cols = sel[:, :, None] + base[None, None, :]     # [B, 8, 26]
        cols = cols.reshape(B, 8 * len(base))            # core-local padded
        valid = cols < NS
        gls.append(c * NS + np.minimum(cols, NS - 1))
        vas.append(valid)
    gi = np.concatenate(gls, axis=1)                     # [B, 8*26*M]
    va = np.concatenate(vas, axis=1)

    V = np.einsum("bkd,bd->bk", fb[gi], fn, optimize=True).astype(np.float32)
    V = np.where(va, V, -np.inf)

    # lax.top_k order: value desc, ties -> lowest original index
    order = np.lexsort((gi, -V.astype(np.float64)), axis=-1)

    # walk to K+1 unique rows (guards duplicate candidates), drop rank 0
    sel_gi = np.empty((B, K), dtype=np.int64)
    for b in range(B):
        got = 0
        prev = -1
        for pos in order[b]:
            g = gi[b, pos]
            if g == prev:
                continue
            prev = g
            if got > 0:
                sel_gi[b, got - 1] = g
            got += 1
            if got == K + 1:
                break

    sbs = sb[sel_gi].astype(np.float64)                  # [B, K, C]
    h = (sbs * np.log(sbs)).sum(-1)
    q = np.einsum("bkc,bc->bk", sbs, p.astype(np.float64))
    kl = (h - q).sum(-1).mean()

    ps = p.astype(np.float64)
    disp = ((ps.sum(0) ** 2).sum() - (ps * ps).sum()) / B
    return np.float32(kl + ALPHA * disp)


def run(inputs, trace=False):
    nc = _get_nc()
    in_maps, fn, fb, sb, p = _prep(**inputs)
    res = run_bass_kernel_spmd(nc, in_maps, list(range(M)), trace=trace)
    return _merge(res.results, fn, fb, sb, p), res


def kernel(features, predictions, fea_bank, score_bank, trg_idx):
    loss, _ = run(
        dict(
            features=features,
            predictions=predictions,
            fea_bank=fea_bank,
            score_bank=score_bank,
            trg_idx=trg_idx,
        )
    )
    return loss
